# revision 3
# baseline (speedup 1.0000x reference)
"""Trainium2 Bass kernel for nn_CompressAttn (compressed-KV sparse attention), v2.

Shapes (hardcoded per spec): B=2, N=4096, QH=32, KH=2, D=128, kernel_size=32,
stride=16 -> M=255 compressed blocks, G=16 query heads per kv head.

Sharding over 8 NeuronCores: core = (b, kv_head, half-of-16-query-heads) --
batch x kv-head x tensor-head parallel, 8 query heads per core; k/v slices
replicated across the 2 cores sharing a kv head; q/out fully partitioned.

v2 design (105.6us v1 baseline -> 77.3us, hardware-validated rel err 5.3e-3
vs the 2e-2 gate):
  * Causal staircase masking moved off DVE entirely: host-built rank-64
    indicator matrices A (one per masked (m-tile, chunk) pair) and a shared
    B in fp8e5m2 are matmul'd in DoubleRow perf mode and ACCUMULATED into
    the score PSUM as a -49152 additive bias; exp then produces exact
    zeros, so no mask tensors, mask multiplies, or pad-column fixups exist.
  * Scores in fp8e4m3 DoubleRow (0.5 PE cycles/column): q ships as fp8
    [64, 2, N] with d-pairs packed per partition; CK is rebuilt on device
    into the same layout via two selector matmuls + an ACT cast. CK is
    pre-scaled by 0.25 for fp8 dynamic range; the exp activation's scale
    operand undoes it (numpy pipeline sim: rel L2 err 5.3e-3).
  * Softmax denominators via 1-column PE matmuls (E_tile^T @ ones),
    essentially free on PE, accumulated into a per-pair strip inside one
    manually-partitioned PSUM bank that also hosts the V-compression
    PSUM and its transpose scratch (region-level dependency tracking +
    carefully placed start_tensor_calc marks keep reuse safe).
  * One reciprocal per exp-group, emitted BEFORE the PV matmuls so it
    executes on DVE while PE streams PVs; normalization is one fused
    broadcast-multiply per 512-query chunk (PSUM [128,4,128] x rec
    [128,4,1] stride-0 AP) written as fp16 directly. All of it on DVE:
    GPSIMD cannot access PSUM on real TRN2 hardware.
  * exp ops merged: chunk pairs (0,1),(2,3) share one [128,1024] PSUM
    tile and one Exp; chunks 4-7 exp both m-tiles in one op: 6 exp ops
    per head. PSUM: 2x2-bank score tiles + 3x1-bank PV tiles + 1 shared
    bank = 8 banks exactly.
  * Output fp16 in partition-major DRAM layout ([128, 4096] contiguous
    8KB rows -> no sub-512B DMA descriptor penalty); host restores layout.
  * Two heads are processed as interleaved streams with a deferred
    post-exp work queue (lookahead) so PE score matmuls of later groups
    precede older groups' PV/den work; staged k/q/v DMA quarters and a
    split two-phase V compression cut the startup from 17us to 7us.

Engine budget per core (cost model): ACT 51.7us (exp; the floor), DVE
50.8 (normalize+recip), DMA 42.5, PE 38.8 (all matmuls), wall 77.3us.
"""

import json
import os
from collections import deque
from contextlib import ExitStack

import ml_dtypes
import numpy as np

import concourse.mybir as mybir
import concourse.tile as tile
from concourse import bacc
from concourse.bass_utils import run_bass_kernel_spmd

B, N, QH, KH, D = 2, 4096, 32, 2, 128
KS, ST = 32, 16
M = (N - KS) // ST + 1  # 255
MP = 256  # m padded to 256 (pad column masked out)
G = QH // KH  # 16
HPC = 8  # query heads per core
NCORES = 8
CHUNK = 512
NCHUNKS = N // CHUNK
NCC = N // 128  # 32 compression chunks
NT = N // 128  # 32 n-tiles of 128 per head
SM_SCALE = D ** -0.5
WBW = 10  # banded-weight window width (even: fp32r ISA restriction)
CKS = 0.25  # ck pre-quantization scale (fp8 dynamic range); undone at exp
BIG = 49152.0  # staircase bias magnitude (exact in fp8e5m2)

# (m_tile, n_chunk) pairs needing the staircase bias, with the column span
# that can contain masked entries: span = 2063 + 2048*mt - 512*c.
MASKED = {(mt, c): min(512, max(16, 2063 + 2048 * mt - 512 * c))
          for mt in (0, 1) for c in range(NCHUNKS)
          if 0 < 2063 + 2048 * mt - 512 * c and (mt, c) != (1, 0) and
          (mt == 0 or c >= 4)}

# exp groups: chunks sharing one [128, 1024] S-PSUM tile and one Exp op.
EXP_GROUPS = [(0, 1), (2, 3), (4,), (5,), (6,), (7,)]

OPTS = {
    "fp8_scores": True,
    "bias_doublerow": True,
    "bufs_sps": 2, "bufs_ops": 3, "bufs_e": 5,
    "bufs_qg": 4, "bufs_og": 4,
    "lookahead": 3,  # (head, group) items of deferred post-exp work
    "tail_nq": 8,      # out-DMA splits for the last two heads
    "ab_early": False,  # blobAB DMA before q1's first part
    "tail_la": 3,      # lookahead cap over the final groups
}
if os.environ.get("K2_OPTS"):
    for _k, _v in json.loads(os.environ["K2_OPTS"]).items():
        OPTS[_k] = frozenset(_v) if isinstance(_v, list) else _v


def _mts_for_chunk(c):
    """m-tile1 has any valid block iff the chunk's max n >= 16*128 + 31."""
    return (0, 1) if c * CHUNK + CHUNK - 1 >= 16 * 128 + (KS - 1) else (0,)


def _wband(w):
    """Banded compression weights: chunk-c matmul does
    CK^T[:, m0(c)+j] += sum_r X[128c+r, :] * wb[c][r, j]. Chunk 0 is emitted
    full-width [128, 256] (zero-padded) so its start=True matmul initializes
    the whole PSUM strip in one consistent accumulation group."""
    r = np.arange(128)
    wb0 = np.zeros((128, MP), np.float32)
    for j in range(MP):
        k0 = r - 16 * j
        sel = (k0 >= 0) & (k0 < KS)
        wb0[sel, j] = w[k0[sel]]
    rest = np.zeros((NCC - 1, 128, WBW), np.float32)
    for c in range(1, NCC):
        m0 = 8 * c - 2
        for j in range(WBW):
            if m0 + j > M - 1:
                continue
            k = r + 32 - 16 * j
            sel = (k >= 0) & (k < KS)
            rest[c - 1, sel, j] = w[k[sel]]
    return np.concatenate([wb0, rest.transpose(1, 0, 2).reshape(128, -1)],
                          axis=1)  # [128, 256 + 31*WBW]


def _amat(mt, c):
    """Rank-64 staircase indicator A [64, 128] for tile (mt, c):
    bias[m, n] = -BIG iff n_loc <= 16*u+14, u = m_glob + 1 - 32c; row m=255
    (the pad column) is forced fully-masked."""
    mg = np.arange(128) + 128 * mt
    u = mg + 1 - 32 * c
    u[mg > M - 1] = 32
    u = np.clip(u, -1, 32)
    r = np.arange(32)[:, None]
    a1 = (u[None, :] >= r + 1).astype(np.float32)
    a2 = (u[None, :] == r).astype(np.float32)
    return np.concatenate([a1, a2], axis=0)


def _bmat():
    """Shared staircase B [64, 512]: rows 0-31 bucket indicators scaled by
    -BIG; rows 32-63 the partial-bucket (n%16 <= 14) variant."""
    n = np.arange(512)
    r = np.arange(32)[:, None]
    b1 = -BIG * (n[None, :] // 16 == r).astype(np.float32)
    b2 = np.where((n[None, :] % 16) <= 14, b1, 0.0)
    return np.concatenate([b1, b2], axis=0)


def _host_prep(w_k, pe_k, w_v, pe_v):
    wsum_k = max(float(np.sum(w_k)), 1e-6)
    wsum_v = max(float(np.sum(w_v)), 1e-6)
    exp_scale = SM_SCALE / wsum_k / CKS
    scv = 1.0 / wsum_v
    blob1 = np.eye(128, dtype=np.float32)
    blob2 = np.stack([
        w_k @ pe_k,                        # ckb (raw, pre-scale)
        w_v @ pe_v,                        # cvb (raw, pre-scale)
        np.full(128, exp_scale, np.float32),
        np.full(128, scv, np.float32),
        np.full(128, np.exp(exp_scale), np.float32),  # pow base for Pool exp
    ], axis=1).astype(np.float32)
    # blob3 (fp16): banded compression weights + d-pair selectors + the f32
    # scale columns and f32r identity bit-packed as f16 pairs (one DMA).
    wbk = _wband(w_k)
    wbv = _wband(w_v)
    sel = np.zeros((128, 2, 64), np.float32)
    for i in range(2):
        sel[np.arange(64) * 2 + i, i, np.arange(64)] = 1.0
    blob3 = np.concatenate([
        wbk.astype(np.float16), wbv.astype(np.float16),
        sel.reshape(128, 128).astype(np.float16),
    ], axis=1)
    blob12 = np.ascontiguousarray(blob2)
    # staircase bias blobs (input-independent): A8 [64, 2, 128] per masked
    # pair (r = 2p+i packing, rank padded 64->128), B8 [64, 2, 512]
    amats = []
    for (mt, c) in sorted(MASKED):
        ap = np.zeros((128, 128), np.float32)
        ap[0:64] = _amat(mt, c)
        amats.append(ap.reshape(64, 2, 128))
    blobA = np.stack(amats, axis=1).reshape(64, -1)  # [64, (pair, i, m)]
    bp = np.zeros((128, 512), np.float32)
    bp[0:64] = _bmat()
    blobB = bp.reshape(64, 1024)
    blobAB = np.concatenate([blobA, blobB], axis=1)
    return {"blob3": np.ascontiguousarray(blob3),
            "blob12": blob12,
            "blob1": np.ascontiguousarray(blob1),
            "blobAB": np.ascontiguousarray(blobAB).astype(ml_dtypes.float8_e5m2)}


def build_program():
    dt = mybir.dt
    f32, f32r, f16 = dt.float32, dt.float32r, dt.float16
    f8e4, f8e5 = dt.float8e4, dt.float8e5
    AF = mybir.ActivationFunctionType
    ALU = mybir.AluOpType
    DR = mybir.MatmulPerfMode.DoubleRow
    fp8 = OPTS["fp8_scores"]
    WBAND = MP + (NCC - 1) * WBW  # per-tensor wband width in blob3
    W3 = 2 * WBAND + 128

    nc = bacc.Bacc("TRN2", target_bir_lowering=False, debug=False,
                   num_devices=NCORES)
    if fp8:
        qD = nc.dram_tensor("q_s", [HPC, 64, 2 * N], f8e4,
                            kind="ExternalInput").ap()
    else:
        qD = nc.dram_tensor("q_s", [HPC, D, N], f16, kind="ExternalInput").ap()
    kD = nc.dram_tensor("k_s", [128, NCC * D], f16, kind="ExternalInput").ap()
    vD = nc.dram_tensor("v_s", [128, NCC * D], f16, kind="ExternalInput").ap()
    b3D = nc.dram_tensor("blob3", [128, W3], f16, kind="ExternalInput").ap()
    b12D = nc.dram_tensor("blob12", [128, 5], f32, kind="ExternalInput").ap()
    b1D = nc.dram_tensor("blob1", [128, 128], f32r, kind="ExternalInput").ap()
    bABD = nc.dram_tensor("blobAB", [64, len(MASKED) * 256 + 1024], f8e5,
                          kind="ExternalInput").ap()
    oD = nc.dram_tensor("out", [HPC, 128, N], f16, kind="ExternalOutput").ap()

    with tile.TileContext(nc) as tc, ExitStack() as ctx:
        res = ctx.enter_context(tc.tile_pool(name="resident", bufs=1))

        blob3 = res.tile([128, W3], f16, tag="blob3")
        blobAB = res.tile([64, len(MASKED) * 256 + 1024], f8e5, tag="blobAB")
        blobA = blobAB[:, 0:len(MASKED) * 256].rearrange(
            "p (k i f) -> p k i f", i=2, f=128)
        blobB = blobAB[:, len(MASKED) * 256:].rearrange(
            "p (i f) -> p i f", i=2)

        blob12 = res.tile([128, 5], f32, tag="blob12")
        blob1 = res.tile([128, 128], f32r, tag="blob1")
        b2v = blob12[:, 0:5]
        ident = blob1[:, 0:128]
        ckb, cvb = b2v[:, 0:1], b2v[:, 1:2]
        exps, scv = b2v[:, 2:3], b2v[:, 3:4]
        ones = res.tile([128, 1], f16, tag="ones")
        nc.vector.memset(ones[:], 1.0)
        a_idx = {mc: i for i, mc in enumerate(sorted(MASKED))}

        def wband_ap(is_k, c):
            base = 0 if is_k else WBAND
            if c == 0:
                return blob3[:, base:base + MP]
            base += MP + (c - 1) * WBW
            return blob3[:, base:base + WBW]

        qg_pool = ctx.enter_context(tc.tile_pool(name="qg", bufs=OPTS["bufs_qg"]))
        qg_tiles = {}

        def prefetch_q(g, sl=None):
            """sl: optional column sub-range to load (staged startup)."""
            if g >= HPC or (g in qg_tiles and sl is None):
                return
            if sl is None:
                sl = slice(0, N)
            if fp8:
                if g in qg_tiles:
                    q_g = qg_tiles[g]
                else:
                    q_g = qg_pool.tile([64, 2, N], f8e4, tag="qg", name="q_g")
                qs = qD[g].rearrange("p (i n) -> p i n", i=2)
                nc.sync.dma_start(out=q_g[:, :, sl], in_=qs[:, :, sl])
            else:
                if g in qg_tiles:
                    q_g = qg_tiles[g]
                else:
                    q_g = qg_pool.tile([128, N], f16, tag="qg", name="q_g")
                nc.sync.dma_start(out=q_g[:, sl], in_=qD[g][:, sl])
            qg_tiles[g] = q_g

        # ---- compression: ckt f16 [128 d, 256 m]; ck8 [64, 2, 256] e4m3;
        # ---- cvp[mt] f16 [128 m, 128 d].
        # DMA order tuned for startup: k quarters -> q0 (chunks 0/1) -> v
        # quarters -> q0 rest -> q1. K compression matmuls chase the k
        # quarters; V compression is emitted from inside the main loop after
        # head 0's first score group so the first exp fires ~10us earlier.
        ckt = res.tile([128, MP], f16, tag="ckt")
        ck8 = (res.tile([64, 2, MP], f8e4, tag="ck8", name="ck8")
               if fp8 else None)
        cvp = [res.tile([128, D], f16, tag=f"cvp{mt}", name=f"cvp{mt}")
               for mt in range(2)]
        cin = ctx.enter_context(tc.tile_pool(name="cin", bufs=1))
        kt = cin.tile([128, NCC, D], f16, tag="xin_k", name="kt")
        kDr = kD.rearrange("p (c d) -> p c d", d=D)
        for j in range(2):
            nc.sync.dma_start(out=kt[:, 16 * j:16 * (j + 1), :],
                              in_=kDr[:, 16 * j:16 * (j + 1), :])
        nc.sync.dma_start(out=blob3[:], in_=b3D[:])
        nc.sync.dma_start(out=blob12[:], in_=b12D[:])
        prefetch_q(0, sl=slice(0, 2048))
        if OPTS["ab_early"]:
            nc.sync.dma_start(out=blobAB[:], in_=bABD[:])
            prefetch_q(1, sl=slice(0, 2048))
        else:
            prefetch_q(1, sl=slice(0, 2048))
            nc.sync.dma_start(out=blobAB[:], in_=bABD[:])
        vt = cin.tile([128, NCC, D], f16, tag="xin_v", name="vt")
        vDr = vD.rearrange("p (c d) -> p c d", d=D)
        nc.sync.dma_start(out=vt[:, 0:17, :], in_=vDr[:, 0:17, :])
        nc.sync.dma_start(out=blob1[:], in_=b1D[:])
        prefetch_q(0, sl=slice(2048, 3072))
        prefetch_q(1, sl=slice(2048, 3072))
        nc.sync.dma_start(out=vt[:, 17:32, :], in_=vDr[:, 17:32, :])
        prefetch_q(0, sl=slice(3072, N))
        prefetch_q(1, sl=slice(3072, N))

        # tiny dummy activation: makes Bacc place the one-time 1.3us Exp
        # table load right after blob3 lands instead of on the critical path
        scr = res.tile([128, 1], f32, tag="scr")
        nc.scalar.activation(scr[:], exps, AF.Exp)

        def emit_compress_mms(ps, xt, is_k):
            for c in range(NCC):
                m0 = 0 if c == 0 else 8 * c - 2
                wid = MP if c == 0 else WBW
                nc.tensor.matmul(
                    ps[:, m0:m0 + wid],
                    lhsT=xt[:, c, :],
                    rhs=wband_ap(is_k, c),
                    start=(c == 0), stop=(c == NCC - 1),
                )

        with tc.tile_pool(name="cps", bufs=1, space="PSUM") as cps:
            ps = cps.tile([128, MP], f32, tag="cp_k", name="ps")
            emit_compress_mms(ps, kt, True)
            nc.vector.tensor_scalar(out=ckt[:], in0=ps[:],
                                    scalar1=ckb, scalar2=CKS,
                                    op0=ALU.add, op1=ALU.mult)
            if fp8:
                cks = cps.tile([64, 2, MP], f32, tag="cks", name="cks")
                for i in range(2):
                    # i=0's start marks the whole 2KB zero-region; i=1
                    # writes into still-pending bytes (reads 0)
                    nc.tensor.matmul(
                        cks[:, i, :],
                        lhsT=blob3[:, 2 * WBAND + 64 * i:
                                   2 * WBAND + 64 * (i + 1)],
                        rhs=ckt[:], start=(i == 0), stop=(i == 1),
                        skip_group_check=True)
                # cast on the (pre-exp idle) ACT engine, off DVE
                nc.scalar.copy(ck8[:], cks[:])

        # ---- main attention loop ----
        og_pool = ctx.enter_context(tc.tile_pool(name="og", bufs=OPTS["bufs_og"]))
        e_pool = ctx.enter_context(tc.tile_pool(name="e", bufs=OPTS["bufs_e"]))
        rec_pool = ctx.enter_context(tc.tile_pool(name="rec", bufs=2))
        s_ps_pool = ctx.enter_context(
            tc.tile_pool(name="sps", bufs=OPTS["bufs_sps"], space="PSUM"))
        o_ps_pool = ctx.enter_context(
            tc.tile_pool(name="ops", bufs=OPTS["bufs_ops"], space="PSUM"))
        den_pool = ctx.enter_context(
            tc.tile_pool(name="den", bufs=1, space="PSUM"))

        # One permanent bank manually partitioned: per-pair denominator
        # strips (alternating halves of cols 0-128), the V-compression PSUM
        # (cols 128-384) and its transpose scratch (cols 384-512). Region-
        # level dependency tracking covers the reuse across pairs; start
        # flags are chosen so a pending-zero bank mark never sits between a
        # write and a dependent accumulate (see den start rule below).
        dv = den_pool.tile([128, 512], f32, tag="dv")
        vps = dv[:, 128:384]
        vtp = dv[:, 384:512]
        cvt = cin.tile([128, MP], f32r, tag="cvt")

        def emit_v_compress(phase):
            """V compression in two phases: chunks 0-16 finalize the mt0
            columns of CV^T (chunk 16's window tops out at m=135), so cvp[0]
            is ready before the second half of v lands."""
            lo, hi = (0, 17) if phase == 0 else (17, NCC)
            for c in range(lo, hi):
                m0 = 0 if c == 0 else 8 * c - 2
                wid = MP if c == 0 else WBW
                nc.tensor.matmul(
                    vps[:, m0:m0 + wid],
                    lhsT=vt[:, c, :],
                    rhs=wband_ap(False, c),
                    start=(c == 0), stop=(c == NCC - 1),
                    skip_group_check=True)
            mt = phase
            nc.vector.tensor_scalar(out=cvt[:, mt * 128:(mt + 1) * 128],
                                    in0=vps[:, mt * 128:(mt + 1) * 128],
                                    scalar1=cvb, scalar2=scv,
                                    op0=ALU.add, op1=ALU.mult)

            # phase 0 writes into still-pending bytes (start=False so the
            # c16/c17 overlap columns keep accumulating); phase 1 must
            # overwrite phase 0's scratch, so start=True re-marks the bank
            # (safe: every other live col there is written-once-then-read).
            nc.tensor.matmul(
                vtp.bitcast(f32r),
                cvt[:, mt * 128:(mt + 1) * 128],
                ident, is_transpose=True, start=(mt == 1), stop=True,
                skip_group_check=True)
            nc.vector.tensor_copy(cvp[mt][:], vtp)

        def score_matmul(s_seg, q_g, mt, c):
            masked = (mt, c) in MASKED
            if fp8:
                nc.tensor.matmul(
                    s_seg,
                    lhsT=ck8[:, :, mt * 128:(mt + 1) * 128],
                    rhs=q_g[:, :, c * CHUNK:(c + 1) * CHUNK],
                    start=True, stop=not masked, perf_mode=DR)
            else:
                nc.tensor.matmul(
                    s_seg,
                    lhsT=ckt[:, mt * 128:(mt + 1) * 128],
                    rhs=q_g[:, c * CHUNK:(c + 1) * CHUNK],
                    start=True, stop=not masked)
            if masked:
                span = MASKED[(mt, c)]
                nc.tensor.matmul(
                    s_seg[:, 0:span],
                    lhsT=blobA[:, a_idx[(mt, c)], :, :],
                    rhs=blobB[:, :, 0:span],
                    start=False, stop=True, perf_mode=DR)

        pending = deque()

        def flush(keep):
            while len(pending) > keep:
                pending.popleft()()

        # Two heads are processed as interleaved streams (A, B): their
        # dependency chains are independent, so B's scores/PVs fill the
        # handoff bubbles of A's score->exp->PV chain and ACT stays busy.
        for p0 in range(0, HPC, 2):
            heads = (p0, p0 + 1)
            for h in heads:
                prefetch_q(h)
            q_gs = {h: qg_tiles.pop(h) for h in heads}
            o_gs = {h: og_pool.tile([128, NT, D], f16, tag="og", name="o_g")
                    for h in heads}
            den_ps = dv[:, 64 * ((p0 // 2) % 2):64 * ((p0 // 2) % 2) + 64]
            rec_sb = rec_pool.tile([128, 8 * NCHUNKS], f32, tag="rec")

            for group in EXP_GROUPS:
                for h in heads:
                    hb = 4 * NCHUNKS * (h - p0)  # den/rec column base
                    q_g, o_g = q_gs[h], o_gs[h]
                    segs = ([(c, 0) for c in group] if len(group) == 2
                            else [(group[0], mt)
                                  for mt in _mts_for_chunk(group[0])])
                    s_ps = s_ps_pool.tile([128, len(segs) * CHUNK], f32,
                                          tag="sps", name="s_ps",
                                          padded_shape=[128, 1024])
                    for si, (c, mt) in enumerate(segs):
                        score_matmul(s_ps[:, si * CHUNK:(si + 1) * CHUNK],
                                     q_g, mt, c)
                    e_sb = e_pool.tile([128, len(segs) * CHUNK], f16, tag="e",
                                       name="e_sb", padded_shape=[128, 1024])
                    nc.scalar.activation(e_sb[:], s_ps[:], AF.Exp, scale=exps)


                    def post_work(h=h, hb=hb, group=group, segs=segs,
                                  e_sb=e_sb, o_g=o_g, den_ps=den_ps,
                                  rec_sb=rec_sb):
                        # dens for all chunks of the group first (tiny PE
                        # ops), then ONE reciprocal for the whole group --
                        # it executes on DVE while PE streams the PV
                        # matmuls, so the norm's only exposed wait is the
                        # last PV's semaphore.
                        e_ts = {}
                        for c in group:
                            smt = [(si, mt)
                                   for si, (cc, mt) in enumerate(segs)
                                   if cc == c]
                            e_ts[c] = smt
                            for t in range(4):
                                for j, (si, mt) in enumerate(smt):
                                    nc.tensor.matmul(
                                        den_ps[:, hb + 4 * c + t:
                                               hb + 4 * c + t + 1],
                                        lhsT=e_sb[:, si * CHUNK + t * 128:
                                                  si * CHUNK + (t + 1) * 128],
                                        rhs=ones[:],
                                        start=(p0 >= 4 and hb == 0 and
                                               c == 0 and t == 0 and j == 0),
                                        stop=(j == len(smt) - 1),
                                        skip_group_check=True)
                        c0, nch = group[0], len(group)
                        dsl = den_ps[:, hb + 4 * c0:hb + 4 * (c0 + nch)]
                        if c0 == 0:
                            # queries n < 31 see no block: denom would be 0
                            nc.vector.tensor_scalar_max(
                                den_ps[:, hb:hb + 4], den_ps[:, hb:hb + 4],
                                1e-30)
                        rsl = rec_sb[:, hb + 4 * c0:hb + 4 * (c0 + nch)]
                        nc.vector.reciprocal(rsl, dsl)
                        o_pss = {}
                        for c in group:
                            o_ps = o_ps_pool.tile([128, CHUNK], f32,
                                                  tag="ops", name="o_ps")
                            o_pss[c] = o_ps
                            for t in range(4):
                                for j, (si, mt) in enumerate(e_ts[c]):
                                    nc.tensor.matmul(
                                        o_ps[:, t * 128:(t + 1) * 128],
                                        lhsT=e_sb[:, si * CHUNK + t * 128:
                                                  si * CHUNK + (t + 1) * 128],
                                        rhs=cvp[mt][:],
                                        start=(t == 0 and j == 0),
                                        stop=(j == len(e_ts[c]) - 1),
                                        skip_group_check=True)
                        for c in group:
                            # GPSIMD cannot touch PSUM on real TRN2: the
                            # fused broadcast normalize lives on DVE
                            nc.vector.tensor_tensor(
                                out=o_g[:, 4 * c:4 * c + 4, :],
                                in0=o_pss[c][:].rearrange(
                                    "p (t d) -> p t d", t=4),
                                in1=rec_sb[:, hb + 4 * c:hb + 4 * c + 4]
                                .unsqueeze(-1).broadcast_to([128, 4, D]),
                                op=ALU.mult)
                        if group[-1] == NCHUNKS - 1:
                            nq = OPTS["tail_nq"] if h >= HPC - 2 else 2
                            for hf in range(nq):
                                nc.sync.dma_start(
                                    out=oD[h, :, hf * (N // nq):
                                           (hf + 1) * (N // nq)],
                                    in_=o_g[:, hf * (NT // nq):
                                            (hf + 1) * (NT // nq), :])

                    pending.append(post_work)
                    if p0 == 0 and h == heads[0]:
                        if group == EXP_GROUPS[1]:
                            emit_v_compress(0)
                        elif group == EXP_GROUPS[3]:
                            emit_v_compress(1)
                    if h == heads[1]:
                        # mid-pair prefetch so the next pair's q lands
                        # before its first score matmuls
                        if group == EXP_GROUPS[2]:
                            prefetch_q(p0 + 2)
                        elif group == EXP_GROUPS[3]:
                            prefetch_q(p0 + 3)
                    la = OPTS["lookahead"]
                    if p0 == HPC - 2 and group[-1] >= 6:
                        la = min(la, OPTS["tail_la"])
                    flush(la)
        flush(0)

    nc.compile()
    return nc


_PROGRAM = None


def _get_program():
    global _PROGRAM
    if _PROGRAM is None:
        _PROGRAM = build_program()
    return _PROGRAM


def _in_maps(q, k, v, w_k, pe_k, w_v, pe_v):
    prep = _host_prep(w_k, pe_k, w_v, pe_v)
    qt = q.transpose(0, 2, 3, 1)  # [B, QH, D, N]
    in_maps = []
    for core in range(NCORES):
        b, h, half = core // 4, (core // 2) % 2, core % 2
        qh0 = h * G + half * HPC
        if OPTS["fp8_scores"]:
            q_s = np.ascontiguousarray(
                qt[b, qh0:qh0 + HPC].reshape(HPC, 64, 2 * N)).astype(
                    ml_dtypes.float8_e4m3fn)
        else:
            q_s = np.ascontiguousarray(qt[b, qh0:qh0 + HPC]).astype(np.float16)
        in_maps.append({
            "q_s": q_s,
            "k_s": np.ascontiguousarray(
                k[b, :, h, :].reshape(NCC, 128, D).transpose(1, 0, 2)
                .reshape(128, NCC * D)).astype(np.float16),
            "v_s": np.ascontiguousarray(
                v[b, :, h, :].reshape(NCC, 128, D).transpose(1, 0, 2)
                .reshape(128, NCC * D)).astype(np.float16),
            **prep,
        })
    return in_maps


def _unshard(results):
    out = np.empty((B, QH, N, D), np.float32)
    for core in range(NCORES):
        b, h, half = core // 4, (core // 2) % 2, core % 2
        qh0 = h * G + half * HPC
        # device layout [HPC, 128 p, N]: col j*128+p holds query n = j*128+p
        o = np.asarray(results[core]["out"], np.float32).reshape(
            HPC, 128, NT, D).transpose(0, 2, 1, 3).reshape(HPC, N, D)
        out[b, qh0:qh0 + HPC] = o
    return np.ascontiguousarray(out.transpose(0, 2, 1, 3))


def kernel(**inputs):
    q = np.asarray(inputs["q"], np.float32)
    k = np.asarray(inputs["k"], np.float32)
    v = np.asarray(inputs["v"], np.float32)
    w_k = np.asarray(inputs["w_k"], np.float32)
    pe_k = np.asarray(inputs["pe_k"], np.float32)
    w_v = np.asarray(inputs["w_v"], np.float32)
    pe_v = np.asarray(inputs["pe_v"], np.float32)
    assert int(inputs["kernel_size"]) == KS and int(inputs["stride"]) == ST
    assert q.shape == (B, N, QH, D) and k.shape == (B, N, KH, D)

    nc = _get_program()
    rr = run_bass_kernel_spmd(nc, _in_maps(q, k, v, w_k, pe_k, w_v, pe_v),
                              list(range(NCORES)))
    return _unshard(rr.results)
